# revision 16
# baseline (speedup 1.0000x reference)
"""Multi-Head Latent Attention (MLA) forward, sharded over 8 Trainium2 cores.

Reference computation (all fp32):
    Q = x @ Wq           (B,T,2048) -> heads (B,H,T,128)
    c = x @ Wdown        (B,T,512)                      [output #2]
    K = c @ Wk_up, V = c @ Wv_up                        (B,H,T,128)
    out = softmax(Q K^T / sqrt(128), causal) @ V
    y = out @ Wo                                        [output #1]

Sharding: core = (b, g) with b in {0,1} batch, g in {0..3} head-group of 4
heads.  Wq/Wk_up/Wv_up are column-sharded, Wo row-sharded; Wdown replicated.
Each core returns a partial y (summed over g on the host) and c^T (taken
from the g=0 core of each batch, transposed on the host).

On-device layout trick: everything is computed "transposed" so that no
on-chip transposes are needed anywhere:
    x^T   (D, T)   supplied pre-transposed by the host
    Q^T   (d, T)   = Wq_g^T x^T      via matmul(lhsT=Wq tile, rhs=x^T tile)
    c^T   (r, T)   = Wdown^T x^T
    K^T   (d, T)   = Wk_g^T c^T
    V     (T, d)   = c Wv_g          via matmul(lhsT=c^T tile, rhs=Wv tile)
    S^T   (k, q)   = K Q^T           via matmul(lhsT=K^T tile, rhs=Q^T tile)
    P~^T  (k, q)   = exp(S^T/sqrt(d))           (ACT engine; no max-sub needed,
                                                 scores are O(1))
    O^T   (d, q)   = V^T P~^T        via matmul(lhsT=V tile, rhs=P~^T tile)
    y     (t, m)   = O Wo_g          via matmul(lhsT=O^T tile, rhs=Wo tile)
Softmax denominators: DVE-accumulate P~^T tiles over k-blocks, then a single
ones-matmul per (head, q-block) reduces the partition axis; normalization is
a per-column multiply of O^T with a GPSIMD partition-broadcast reciprocal.

Causality: S^T/PV tiles are skipped at 512-query granularity; the diagonal
band is masked exactly with 4 precomputed 0/1 masks (DVE multiply).
"""

import os
import sys

import numpy as np

for _p in ("/opt/trn_rl_repo",):
    if _p not in sys.path and os.path.isdir(_p):
        sys.path.insert(0, _p)

import concourse.bacc as bacc
import concourse.bass as bass
import concourse.mybir as mybir
import concourse.tile as tile
from concourse.bass_utils import run_bass_kernel_spmd

F32 = mybir.dt.float32

B = 2
T = 2048
D = 2048
H_TOT = 16
DH = 128
R = 512  # kv lora rank
G = 4  # head groups (cores per batch)
HPG = 4  # heads per group
GC = HPG * DH  # 512 columns per group
P = 128
NT = T // 512  # 4 query/time column-blocks of 512
NDB = D // P  # 16 contraction blocks over d_model
NRB = R // P  # 4 contraction blocks over rank
NKB = T // P  # 16 key blocks of 128
SCALE = float(DH) ** -0.5

# Matmul dtype: float32r == TF32 on the PE array (1 cycle/row at free dim
# >= 256 vs 4 cycles/row for fp32).  Flip to mybir.dt.float32 for full
# precision at 4x the PE time.
MM_DT = mybir.dt.float32r


def _mm(nc, out, lhsT, rhs, start, stop):
    nc.tensor.matmul(out, lhsT, rhs, start=start, stop=stop)


def build_mla_kernel():
    # Bacc (not raw Bass): its compile() pass legalizes sync waits to the
    # TRN2 limit (1 wait/instruction) and auto-inserts gpsimd library /
    # ACT table loads — walrus rejects the raw Tile output otherwise.
    nc = bacc.Bacc("TRN2", target_bir_lowering=False, debug=False)

    xT = nc.dram_tensor("xt", [D, T], MM_DT, kind="ExternalInput").ap()
    wq = nc.dram_tensor("wq", [D, GC], MM_DT, kind="ExternalInput").ap()
    wdown = nc.dram_tensor("wdown", [D, R], MM_DT, kind="ExternalInput").ap()
    wk = nc.dram_tensor("wk", [R, GC], MM_DT, kind="ExternalInput").ap()
    wv = nc.dram_tensor("wv", [R, GC], MM_DT, kind="ExternalInput").ap()
    wo = nc.dram_tensor("wo", [GC, D], MM_DT, kind="ExternalInput").ap()
    masks = nc.dram_tensor("masks", [G, P, 512], MM_DT, kind="ExternalInput").ap()
    ones_in = nc.dram_tensor("ones", [P, 1], MM_DT, kind="ExternalInput").ap()
    ident_in = nc.dram_tensor("ident", [P, P], MM_DT, kind="ExternalInput").ap()
    y = nc.dram_tensor("y", [T, D], F32, kind="ExternalOutput").ap()
    cT = nc.dram_tensor("ct", [R, T], MM_DT, kind="ExternalOutput").ap()

    with tile.TileContext(nc) as tc:
        _emit(nc, tc, xT, wq, wdown, wk, wv, wo, masks, ones_in, ident_in, y, cT)
    nc.compile()
    return nc


def _emit(nc, tc, xT, wq, wdown, wk, wv, wo, masks, ones_in, ident_in, y, cT):
    from contextlib import ExitStack

    ctx = ExitStack()
    with ctx:
        consts = ctx.enter_context(tc.tile_pool(name="consts", bufs=1))
        ones = consts.tile([P, 1], MM_DT)
        nc.sync.dma_start(out=ones[:], in_=ones_in[:])
        mask_t = []
        for j in range(4):
            mt = consts.tile([P, 512], MM_DT, name=f"mask{j}", tag=f"mask{j}")
            nc.sync.dma_start(out=mt[:], in_=masks[j])
            mask_t.append(mt)
        ident = consts.tile([P, P], MM_DT)
        nc.sync.dma_start(out=ident[:], in_=ident_in[:])

        # Resident activations (live through attention).
        qt_pool = ctx.enter_context(tc.tile_pool(name="qt", bufs=1))
        kt_pool = ctx.enter_context(tc.tile_pool(name="kt", bufs=1))
        v_pool = ctx.enter_context(tc.tile_pool(name="vres", bufs=1))
        QT = [qt_pool.tile([P, T], MM_DT, name=f"qt{i}", tag=f"qt{i}") for i in range(HPG)]
        KT = [kt_pool.tile([P, T], MM_DT, name=f"kt{i}", tag=f"kt{i}") for i in range(HPG)]
        V = [v_pool.tile([P, GC], MM_DT, name=f"v{i}", tag=f"v{i}") for i in range(NKB)]

        # ---------------- Phase AB: projections ----------------
        with (
            tc.tile_pool(name="wq_res", bufs=1) as wq_pool,
            tc.tile_pool(name="wkv_res", bufs=1) as wkv_pool,
            tc.tile_pool(name="wd_str", bufs=4) as wd_pool,
            tc.tile_pool(name="xt_str", bufs=4) as xt_pool,
            tc.tile_pool(name="ct_str", bufs=6) as ct_pool,
            tc.tile_pool(name="ps_ab", bufs=8, space="PSUM") as ps_ab,
        ):
            wq_t = [
                wq_pool.tile([P, GC], MM_DT, name=f"wqt{i}", tag=f"wqt{i}")
                for i in range(NDB)
            ]
            wk_t = [
                wkv_pool.tile([P, GC], MM_DT, name=f"wkt{i}", tag=f"wkt{i}")
                for i in range(NRB)
            ]
            wv_t = [
                wkv_pool.tile([P, GC], MM_DT, name=f"wvt{i}", tag=f"wvt{i}")
                for i in range(NRB)
            ]

            for tb in range(NT):
                ts = slice(tb * 512, (tb + 1) * 512)
                # Accumulate Q^T (4 head tiles) and c^T (4 rank tiles) for
                # this t-block; one pass over the 16 D-blocks of x^T.
                ps_q = [
                    ps_ab.tile([P, 512], F32, name=f"psq{tb}_{i}", tag="ps_ab")
                    for i in range(HPG)
                ]
                ps_c = [
                    ps_ab.tile([P, 512], F32, name=f"psc{tb}_{i}", tag="ps_ab")
                    for i in range(NRB)
                ]
                for db in range(NDB):
                    xt_tile = xt_pool.tile([P, 512], MM_DT, name="xt_tile")
                    nc.sync.dma_start(out=xt_tile[:], in_=xT[db * P : (db + 1) * P, ts])
                    if tb == 0:
                        nc.sync.dma_start(
                            out=wq_t[db][:], in_=wq[db * P : (db + 1) * P, :]
                        )
                    wd_tile = wd_pool.tile([P, R], MM_DT, name="wd_tile")
                    nc.sync.dma_start(
                        out=wd_tile[:], in_=wdown[db * P : (db + 1) * P, :]
                    )
                    first, last = db == 0, db == NDB - 1
                    for i in range(HPG):
                        _mm(
                            nc,
                            ps_q[i][:],
                            wq_t[db][:, i * P : (i + 1) * P],
                            xt_tile[:],
                            first,
                            last,
                        )
                    for i in range(NRB):
                        _mm(
                            nc,
                            ps_c[i][:],
                            wd_tile[:, i * P : (i + 1) * P],
                            xt_tile[:],
                            first,
                            last,
                        )
                if tb == 0:
                    for i in range(NRB):
                        nc.sync.dma_start(out=wk_t[i][:], in_=wk[i * P : (i + 1) * P, :])
                        nc.sync.dma_start(out=wv_t[i][:], in_=wv[i * P : (i + 1) * P, :])
                ct_t = []
                for i in range(HPG):
                    nc.vector.tensor_copy(QT[i][:, ts], ps_q[i][:])
                for i in range(NRB):
                    ct_tile = ct_pool.tile([P, 512], MM_DT, name="ct_tile")
                    nc.vector.tensor_copy(ct_tile[:], ps_c[i][:])
                    nc.sync.dma_start(out=cT[i * P : (i + 1) * P, ts], in_=ct_tile[:])
                    ct_t.append(ct_tile)

                # K^T and V for this t-block from c^T.
                ps_k = [
                    ps_ab.tile([P, 512], F32, name=f"psk{tb}_{i}", tag="ps_ab")
                    for i in range(HPG)
                ]
                ps_v = [
                    ps_ab.tile([P, 512], F32, name=f"psv{tb}_{i}", tag="ps_ab")
                    for i in range(4)
                ]
                for rb in range(NRB):
                    first, last = rb == 0, rb == NRB - 1
                    for i in range(HPG):
                        _mm(
                            nc,
                            ps_k[i][:],
                            wk_t[rb][:, i * P : (i + 1) * P],
                            ct_t[rb][:],
                            first,
                            last,
                        )
                    for i in range(4):
                        _mm(
                            nc,
                            ps_v[i][:],
                            ct_t[rb][:, i * P : (i + 1) * P],
                            wv_t[rb][:],
                            first,
                            last,
                        )
                for i in range(HPG):
                    nc.vector.tensor_copy(KT[i][:, ts], ps_k[i][:])
                for i in range(4):
                    nc.vector.tensor_copy(V[tb * 4 + i][:], ps_v[i][:])

        # ---------------- Phase C/D: attention + output proj ----------------
        with (
            tc.tile_pool(name="wo_res", bufs=1) as wo_pool,
            tc.tile_pool(name="ot", bufs=9) as ot_pool,
            tc.tile_pool(name="pexp", bufs=6) as pexp_pool,
            tc.tile_pool(name="acc", bufs=4) as acc_pool,
            tc.tile_pool(name="rec", bufs=3) as rec_pool,
            tc.tile_pool(name="rbc", bufs=2) as rbc_pool,
            tc.tile_pool(name="ysb", bufs=4) as ysb_pool,
            tc.tile_pool(name="ps_s", bufs=2, space="PSUM") as ps_s,
            tc.tile_pool(name="ps_o", bufs=2, space="PSUM") as ps_o,
            tc.tile_pool(name="ps_misc", bufs=2, space="PSUM") as ps_misc,
        ):
            wo_t = [
                wo_pool.tile([P, D], MM_DT, name=f"wot{i}", tag=f"wot{i}")
                for i in range(HPG)
            ]
            for i in range(HPG):
                nc.sync.dma_start(out=wo_t[i][:], in_=wo[i * P : (i + 1) * P, :])

            for qb in range(NT):
                qs = slice(qb * 512, (qb + 1) * 512)
                nkb = 4 * qb + 4  # key blocks 0..nkb-1 are (at least partly) live
                ot_t = []
                for h in range(HPG):
                    ps_ot = ps_o.tile([P, 512], F32, name=f"pso{qb}_{h}", tag="ps_o")
                    acc = acc_pool.tile([P, 512], MM_DT, name="acc")
                    # key blocks in pairs -> one 2-bank S^T psum + one exp
                    for kp in range(nkb // 2):
                        ps_st = ps_s.tile([P, 1024], F32, name="ps_st", tag="ps_s")
                        pexp = pexp_pool.tile([P, 1024], MM_DT, name="pexp")
                        for half in range(2):
                            kb = 2 * kp + half
                            j = kb - 4 * qb
                            sl = ps_st[:, half * 512 : (half + 1) * 512]
                            _mm(
                                nc,
                                sl,
                                KT[h][:, kb * P : (kb + 1) * P],
                                QT[h][:, qs],
                                True,
                                j < 0,
                            )
                            if j >= 0:  # diagonal: add -1e9 mask via PE
                                _mm(nc, sl, ident[:], mask_t[j][:], False, True)
                        nc.scalar.activation(
                            pexp[:],
                            ps_st[:],
                            mybir.ActivationFunctionType.Exp,
                            scale=SCALE,
                        )
                        for half in range(2):
                            kb = 2 * kp + half
                            ph = pexp[:, half * 512 : (half + 1) * 512]
                            eng = nc.gpsimd if qb % 2 == 1 else nc.vector
                            if kb == 0:
                                eng.tensor_copy(acc[:], ph)
                            else:
                                eng.tensor_add(acc[:], acc[:], ph)
                            _mm(
                                nc,
                                ps_ot[:],
                                V[kb][:, h * P : (h + 1) * P],
                                ph,
                                kb == 0,
                                kb == nkb - 1,
                            )
                    # softmax denominators: partition-reduce acc with ones
                    ps_den = ps_misc.tile([1, 512], F32, name="ps_den", tag="ps_misc")
                    _mm(nc, ps_den[:], ones[:], acc[:], True, True)
                    rec = rec_pool.tile([1, 512], F32, name="rec")
                    nc.vector.reciprocal_approx_fast(rec[:], ps_den[:])
                    rbc = rbc_pool.tile([P, 512], F32, name="rbc")
                    nc.gpsimd.partition_broadcast(rbc[:], rec[:])
                    ot = ot_pool.tile([P, 512], MM_DT, name="ot")
                    nc.vector.tensor_mul(ot[:], ps_ot[:], rbc[:])
                    ot_t.append(ot)

                # y rows for this q-block: y[t, :] = sum_h O_h[t, :] @ Wo[h]
                for tt in range(4):
                    trow = qb * 512 + tt * P
                    for mb in range(NT):
                        ms = slice(mb * 512, (mb + 1) * 512)
                        ps_yt = ps_misc.tile([P, 512], F32, name="ps_yt", tag="ps_misc")
                        for h in range(HPG):
                            _mm(
                                nc,
                                ps_yt[:],
                                ot_t[h][:, tt * P : (tt + 1) * P],
                                wo_t[h][:, ms],
                                h == 0,
                                h == HPG - 1,
                            )
                        ysb = ysb_pool.tile([P, 512], F32, name="ysb")
                        nc.vector.tensor_copy(ysb[:], ps_yt[:])
                        nc.sync.dma_start(out=y[trow : trow + P, ms], in_=ysb[:])


def _build_masks():
    ki = np.arange(P)[:, None]
    qi = np.arange(512)[None, :]
    return np.stack(
        [np.where(qi >= ki + 128 * j, 0.0, -1e9).astype(np.float32) for j in range(4)],
        axis=0,
    )


_NC_CACHE = None


def _get_nc():
    global _NC_CACHE
    if _NC_CACHE is None:
        _NC_CACHE = build_mla_kernel()
    return _NC_CACHE


def kernel(x, Wq, Wdown, Wk_up, Wv_up, Wo, _trace=False):
    x = np.asarray(x, dtype=np.float32)
    Wq = np.asarray(Wq, dtype=np.float32)
    Wdown = np.asarray(Wdown, dtype=np.float32)
    Wk_up = np.asarray(Wk_up, dtype=np.float32)
    Wv_up = np.asarray(Wv_up, dtype=np.float32)
    Wo = np.asarray(Wo, dtype=np.float32)

    nc = _get_nc()
    masks = _build_masks()
    xTs = [np.ascontiguousarray(x[b].T) for b in range(B)]
    in_maps = []
    for core in range(8):
        b, g = divmod(core, G)
        gs = slice(g * GC, (g + 1) * GC)
        in_maps.append(
            {
                "xt": xTs[b],
                "wq": np.ascontiguousarray(Wq[:, gs]),
                "wdown": Wdown,
                "wk": np.ascontiguousarray(Wk_up[:, gs]),
                "wv": np.ascontiguousarray(Wv_up[:, gs]),
                "wo": np.ascontiguousarray(Wo[gs, :]),
                "masks": masks,
                "ones": np.ones((P, 1), dtype=np.float32),
                "ident": np.eye(P, dtype=np.float32),
            }
        )
    res = run_bass_kernel_spmd(nc, in_maps, list(range(8)), trace=_trace)
    outs = res.results
    y = np.stack(
        [
            outs[b * G]["y"]
            + outs[b * G + 1]["y"]
            + outs[b * G + 2]["y"]
            + outs[b * G + 3]["y"]
            for b in range(B)
        ],
        axis=0,
    )
    c = np.stack([np.ascontiguousarray(outs[b * G]["ct"].T) for b in range(B)], axis=0)
    if _trace:
        return (y, c), res
    return y, c


# revision 17
# speedup vs baseline: 1.4791x; 1.4791x over previous
"""Multi-Head Latent Attention (MLA) forward, sharded over 8 Trainium2 cores.

Reference computation (all fp32):
    Q = x @ Wq           (B,T,2048) -> heads (B,H,T,128)
    c = x @ Wdown        (B,T,512)                      [output #2]
    K = c @ Wk_up, V = c @ Wv_up                        (B,H,T,128)
    out = softmax(Q K^T / sqrt(128), causal) @ V
    y = out @ Wo                                        [output #1]

Sharding: core = (b, g) with b in {0,1} batch, g in {0..3} head-group of 4
heads.  Wq/Wk_up/Wv_up are column-sharded, Wo row-sharded; Wdown replicated.
Each core returns a partial y (summed over g on the host) and c^T (taken
from the g=0 core of each batch, transposed on the host).

On-device layout trick: everything is computed "transposed" so that no
on-chip transposes are needed anywhere:
    x^T   (D, T)   supplied pre-transposed by the host
    Q^T   (d, T)   = Wq_g^T x^T      via matmul(lhsT=Wq tile, rhs=x^T tile)
    c^T   (r, T)   = Wdown^T x^T
    K^T   (d, T)   = Wk_g^T c^T
    V     (T, d)   = c Wv_g          via matmul(lhsT=c^T tile, rhs=Wv tile)
    S^T   (k, q)   = K Q^T           via matmul(lhsT=K^T tile, rhs=Q^T tile)
    P~^T  (k, q)   = exp(S^T/sqrt(d))           (ACT engine; no max-sub needed,
                                                 scores are O(1))
    O^T   (d, q)   = V^T P~^T        via matmul(lhsT=V tile, rhs=P~^T tile)
    y     (t, m)   = O Wo_g          via matmul(lhsT=O^T tile, rhs=Wo tile)
Softmax denominators: DVE-accumulate P~^T tiles over k-blocks, then a single
ones-matmul per (head, q-block) reduces the partition axis; normalization is
a per-column multiply of O^T with a GPSIMD partition-broadcast reciprocal.

Causality: S^T/PV tiles are skipped at 512-query granularity; the diagonal
band is masked exactly with 4 precomputed 0/1 masks (DVE multiply).
"""

import os
import sys

import numpy as np

for _p in ("/opt/trn_rl_repo",):
    if _p not in sys.path and os.path.isdir(_p):
        sys.path.insert(0, _p)

import concourse.bacc as bacc
import concourse.bass as bass
import concourse.mybir as mybir
import concourse.tile as tile
from concourse.bass_utils import run_bass_kernel_spmd

F32 = mybir.dt.float32

B = 2
T = 2048
D = 2048
H_TOT = 16
DH = 128
R = 512  # kv lora rank
G = 4  # head groups (cores per batch)
HPG = 4  # heads per group
GC = HPG * DH  # 512 columns per group
P = 128
NT = T // 512  # 4 query/time column-blocks of 512
NDB = D // P  # 16 contraction blocks over d_model
NRB = R // P  # 4 contraction blocks over rank
NKB = T // P  # 16 key blocks of 128
SCALE = float(DH) ** -0.5

# Matmul dtype: float32r == TF32 on the PE array (1 cycle/row at free dim
# >= 256 vs 4 cycles/row for fp32).  Flip to mybir.dt.float32 for full
# precision at 4x the PE time.
MM_DT = mybir.dt.float32r


def _mm(nc, out, lhsT, rhs, start, stop):
    nc.tensor.matmul(out, lhsT, rhs, start=start, stop=stop)


def build_mla_kernel():
    # Bacc (not raw Bass): its compile() pass legalizes sync waits to the
    # TRN2 limit (1 wait/instruction) and auto-inserts gpsimd library /
    # ACT table loads — walrus rejects the raw Tile output otherwise.
    nc = bacc.Bacc("TRN2", target_bir_lowering=False, debug=False)

    xT = nc.dram_tensor("xt", [D, T], MM_DT, kind="ExternalInput").ap()
    wq = nc.dram_tensor("wq", [D, GC], MM_DT, kind="ExternalInput").ap()
    wdown = nc.dram_tensor("wdown", [D, R], MM_DT, kind="ExternalInput").ap()
    wk = nc.dram_tensor("wk", [R, GC], MM_DT, kind="ExternalInput").ap()
    wv = nc.dram_tensor("wv", [R, GC], MM_DT, kind="ExternalInput").ap()
    wo = nc.dram_tensor("wo", [GC, D], MM_DT, kind="ExternalInput").ap()
    masks = nc.dram_tensor("masks", [G, P, 512], MM_DT, kind="ExternalInput").ap()
    ones_in = nc.dram_tensor("ones", [P, 1], MM_DT, kind="ExternalInput").ap()
    ident_in = nc.dram_tensor("ident", [P, P], MM_DT, kind="ExternalInput").ap()
    y = nc.dram_tensor("y", [T, D], F32, kind="ExternalOutput").ap()
    cT = nc.dram_tensor("ct", [R, T], MM_DT, kind="ExternalOutput").ap()

    with tile.TileContext(nc) as tc:
        _emit(nc, tc, xT, wq, wdown, wk, wv, wo, masks, ones_in, ident_in, y, cT)
    nc.compile()
    return nc


def _emit(nc, tc, xT, wq, wdown, wk, wv, wo, masks, ones_in, ident_in, y, cT):
    from contextlib import ExitStack

    ctx = ExitStack()
    with ctx:
        consts = ctx.enter_context(tc.tile_pool(name="consts", bufs=1))
        ones = consts.tile([P, 1], MM_DT)
        nc.sync.dma_start(out=ones[:], in_=ones_in[:])
        mask_t = []
        for j in range(4):
            mt = consts.tile([P, 512], MM_DT, name=f"mask{j}", tag=f"mask{j}")
            nc.sync.dma_start(out=mt[:], in_=masks[j])
            mask_t.append(mt)
        ident = consts.tile([P, P], MM_DT)
        nc.sync.dma_start(out=ident[:], in_=ident_in[:])

        # Resident activations (live through attention).
        qt_pool = ctx.enter_context(tc.tile_pool(name="qt", bufs=1))
        kt_pool = ctx.enter_context(tc.tile_pool(name="kt", bufs=1))
        v_pool = ctx.enter_context(tc.tile_pool(name="vres", bufs=1))
        QT = [qt_pool.tile([P, T], MM_DT, name=f"qt{i}", tag=f"qt{i}") for i in range(HPG)]
        KT = [kt_pool.tile([P, T], MM_DT, name=f"kt{i}", tag=f"kt{i}") for i in range(HPG)]
        V = [v_pool.tile([P, GC], MM_DT, name=f"v{i}", tag=f"v{i}") for i in range(NKB)]

        # ---------------- Phase AB: projections ----------------
        with (
            tc.tile_pool(name="wq_res", bufs=1) as wq_pool,
            tc.tile_pool(name="wkv_res", bufs=1) as wkv_pool,
            tc.tile_pool(name="wd_str", bufs=4) as wd_pool,
            tc.tile_pool(name="xt_str", bufs=4) as xt_pool,
            tc.tile_pool(name="ct_str", bufs=6) as ct_pool,
            tc.tile_pool(name="ps_ab", bufs=8, space="PSUM") as ps_ab,
        ):
            wq_t = [
                wq_pool.tile([P, GC], MM_DT, name=f"wqt{i}", tag=f"wqt{i}")
                for i in range(NDB)
            ]
            wk_t = [
                wkv_pool.tile([P, GC], MM_DT, name=f"wkt{i}", tag=f"wkt{i}")
                for i in range(NRB)
            ]
            wv_t = [
                wkv_pool.tile([P, GC], MM_DT, name=f"wvt{i}", tag=f"wvt{i}")
                for i in range(NRB)
            ]

            for tb in range(NT):
                ts = slice(tb * 512, (tb + 1) * 512)
                # Accumulate Q^T (4 head tiles) and c^T (4 rank tiles) for
                # this t-block; one pass over the 16 D-blocks of x^T.
                ps_q = [
                    ps_ab.tile([P, 512], F32, name=f"psq{tb}_{i}", tag="ps_ab")
                    for i in range(HPG)
                ]
                ps_c = [
                    ps_ab.tile([P, 512], F32, name=f"psc{tb}_{i}", tag="ps_ab")
                    for i in range(NRB)
                ]
                for db in range(NDB):
                    xt_tile = xt_pool.tile([P, 512], MM_DT, name="xt_tile")
                    nc.sync.dma_start(out=xt_tile[:], in_=xT[db * P : (db + 1) * P, ts])
                    if tb == 0:
                        nc.sync.dma_start(
                            out=wq_t[db][:], in_=wq[db * P : (db + 1) * P, :]
                        )
                    wd_tile = wd_pool.tile([P, R], MM_DT, name="wd_tile")
                    nc.sync.dma_start(
                        out=wd_tile[:], in_=wdown[db * P : (db + 1) * P, :]
                    )
                    first, last = db == 0, db == NDB - 1
                    for i in range(HPG):
                        _mm(
                            nc,
                            ps_q[i][:],
                            wq_t[db][:, i * P : (i + 1) * P],
                            xt_tile[:],
                            first,
                            last,
                        )
                    for i in range(NRB):
                        _mm(
                            nc,
                            ps_c[i][:],
                            wd_tile[:, i * P : (i + 1) * P],
                            xt_tile[:],
                            first,
                            last,
                        )
                if tb == 0:
                    for i in range(NRB):
                        nc.sync.dma_start(out=wk_t[i][:], in_=wk[i * P : (i + 1) * P, :])
                        nc.sync.dma_start(out=wv_t[i][:], in_=wv[i * P : (i + 1) * P, :])
                ct_t = []
                for i in range(HPG):
                    nc.vector.tensor_copy(QT[i][:, ts], ps_q[i][:])
                for i in range(NRB):
                    ct_tile = ct_pool.tile([P, 512], MM_DT, name="ct_tile")
                    nc.vector.tensor_copy(ct_tile[:], ps_c[i][:])
                    nc.sync.dma_start(out=cT[i * P : (i + 1) * P, ts], in_=ct_tile[:])
                    ct_t.append(ct_tile)

                # K^T and V for this t-block from c^T.
                ps_k = [
                    ps_ab.tile([P, 512], F32, name=f"psk{tb}_{i}", tag="ps_ab")
                    for i in range(HPG)
                ]
                ps_v = [
                    ps_ab.tile([P, 512], F32, name=f"psv{tb}_{i}", tag="ps_ab")
                    for i in range(4)
                ]
                for rb in range(NRB):
                    first, last = rb == 0, rb == NRB - 1
                    for i in range(HPG):
                        _mm(
                            nc,
                            ps_k[i][:],
                            wk_t[rb][:, i * P : (i + 1) * P],
                            ct_t[rb][:],
                            first,
                            last,
                        )
                    for i in range(4):
                        _mm(
                            nc,
                            ps_v[i][:],
                            ct_t[rb][:, i * P : (i + 1) * P],
                            wv_t[rb][:],
                            first,
                            last,
                        )
                for i in range(HPG):
                    nc.vector.tensor_copy(KT[i][:, ts], ps_k[i][:])
                for i in range(4):
                    nc.vector.tensor_copy(V[tb * 4 + i][:], ps_v[i][:])

        # ---------------- Phase C/D: attention + output proj ----------------
        with (
            tc.tile_pool(name="wo_res", bufs=1) as wo_pool,
            tc.tile_pool(name="ot", bufs=9) as ot_pool,
            tc.tile_pool(name="pexp", bufs=6) as pexp_pool,
            tc.tile_pool(name="acc", bufs=4) as acc_pool,
            tc.tile_pool(name="rec", bufs=3) as rec_pool,
            tc.tile_pool(name="rbc", bufs=2) as rbc_pool,
            tc.tile_pool(name="ysb", bufs=4) as ysb_pool,
            tc.tile_pool(name="ps_s", bufs=2, space="PSUM") as ps_s,
            tc.tile_pool(name="ps_o", bufs=2, space="PSUM") as ps_o,
            tc.tile_pool(name="ps_misc", bufs=2, space="PSUM") as ps_misc,
        ):
            wo_t = [
                wo_pool.tile([P, D], MM_DT, name=f"wot{i}", tag=f"wot{i}")
                for i in range(HPG)
            ]
            for i in range(HPG):
                nc.sync.dma_start(out=wo_t[i][:], in_=wo[i * P : (i + 1) * P, :])

            for qb in range(NT):
                qs = slice(qb * 512, (qb + 1) * 512)
                nkb = 4 * qb + 4  # key blocks 0..nkb-1 are (at least partly) live
                ot_t = []
                for h in range(HPG):
                    ps_ot = ps_o.tile([P, 512], F32, name=f"pso{qb}_{h}", tag="ps_o")
                    acc = acc_pool.tile([P, 512], MM_DT, name="acc")
                    # key blocks in pairs -> one 2-bank S^T psum + one exp
                    for kp in range(nkb // 2):
                        ps_st = ps_s.tile([P, 1024], F32, name="ps_st", tag="ps_s")
                        pexp = pexp_pool.tile([P, 1024], MM_DT, name="pexp")
                        for half in range(2):
                            kb = 2 * kp + half
                            j = kb - 4 * qb
                            sl = ps_st[:, half * 512 : (half + 1) * 512]
                            _mm(
                                nc,
                                sl,
                                KT[h][:, kb * P : (kb + 1) * P],
                                QT[h][:, qs],
                                True,
                                j < 0,
                            )
                            if j >= 0:  # diagonal: add -1e9 mask via PE
                                _mm(nc, sl, ident[:], mask_t[j][:], False, True)
                        nc.scalar.activation(
                            pexp[:],
                            ps_st[:],
                            mybir.ActivationFunctionType.Exp,
                            scale=SCALE,
                        )
                        for half in range(2):
                            kb = 2 * kp + half
                            ph = pexp[:, half * 512 : (half + 1) * 512]
                            if kb == 0:
                                nc.vector.tensor_copy(acc[:], ph)
                            else:
                                nc.vector.tensor_add(acc[:], acc[:], ph)
                            _mm(
                                nc,
                                ps_ot[:],
                                V[kb][:, h * P : (h + 1) * P],
                                ph,
                                kb == 0,
                                kb == nkb - 1,
                            )
                    # softmax denominators: partition-reduce acc with ones
                    ps_den = ps_misc.tile([1, 512], F32, name="ps_den", tag="ps_misc")
                    _mm(nc, ps_den[:], ones[:], acc[:], True, True)
                    rec = rec_pool.tile([1, 512], F32, name="rec")
                    nc.vector.reciprocal_approx_fast(rec[:], ps_den[:])
                    rbc = rbc_pool.tile([P, 512], F32, name="rbc")
                    nc.gpsimd.partition_broadcast(rbc[:], rec[:])
                    ot = ot_pool.tile([P, 512], MM_DT, name="ot")
                    nc.vector.tensor_mul(ot[:], ps_ot[:], rbc[:])
                    ot_t.append(ot)

                # y rows for this q-block: y[t, :] = sum_h O_h[t, :] @ Wo[h]
                for tt in range(4):
                    trow = qb * 512 + tt * P
                    for mb in range(NT):
                        ms = slice(mb * 512, (mb + 1) * 512)
                        ps_yt = ps_misc.tile([P, 512], F32, name="ps_yt", tag="ps_misc")
                        for h in range(HPG):
                            _mm(
                                nc,
                                ps_yt[:],
                                ot_t[h][:, tt * P : (tt + 1) * P],
                                wo_t[h][:, ms],
                                h == 0,
                                h == HPG - 1,
                            )
                        ysb = ysb_pool.tile([P, 512], F32, name="ysb")
                        nc.vector.tensor_copy(ysb[:], ps_yt[:])
                        nc.sync.dma_start(out=y[trow : trow + P, ms], in_=ysb[:])


def _build_masks():
    ki = np.arange(P)[:, None]
    qi = np.arange(512)[None, :]
    return np.stack(
        [np.where(qi >= ki + 128 * j, 0.0, -1e9).astype(np.float32) for j in range(4)],
        axis=0,
    )


_NC_CACHE = None


def _get_nc():
    global _NC_CACHE
    if _NC_CACHE is None:
        _NC_CACHE = build_mla_kernel()
    return _NC_CACHE


def kernel(x, Wq, Wdown, Wk_up, Wv_up, Wo, _trace=False):
    x = np.asarray(x, dtype=np.float32)
    Wq = np.asarray(Wq, dtype=np.float32)
    Wdown = np.asarray(Wdown, dtype=np.float32)
    Wk_up = np.asarray(Wk_up, dtype=np.float32)
    Wv_up = np.asarray(Wv_up, dtype=np.float32)
    Wo = np.asarray(Wo, dtype=np.float32)

    nc = _get_nc()
    masks = _build_masks()
    xTs = [np.ascontiguousarray(x[b].T) for b in range(B)]
    in_maps = []
    for core in range(8):
        b, g = divmod(core, G)
        gs = slice(g * GC, (g + 1) * GC)
        in_maps.append(
            {
                "xt": xTs[b],
                "wq": np.ascontiguousarray(Wq[:, gs]),
                "wdown": Wdown,
                "wk": np.ascontiguousarray(Wk_up[:, gs]),
                "wv": np.ascontiguousarray(Wv_up[:, gs]),
                "wo": np.ascontiguousarray(Wo[gs, :]),
                "masks": masks,
                "ones": np.ones((P, 1), dtype=np.float32),
                "ident": np.eye(P, dtype=np.float32),
            }
        )
    res = run_bass_kernel_spmd(nc, in_maps, list(range(8)), trace=_trace)
    outs = res.results
    y = np.stack(
        [
            outs[b * G]["y"]
            + outs[b * G + 1]["y"]
            + outs[b * G + 2]["y"]
            + outs[b * G + 3]["y"]
            for b in range(B)
        ],
        axis=0,
    )
    c = np.stack([np.ascontiguousarray(outs[b * G]["ct"].T) for b in range(B)], axis=0)
    if _trace:
        return (y, c), res
    return y, c
